# revision 1
# baseline (speedup 1.0000x reference)
"""Masked (expander) linear layer on 8 Trainium2 NeuronCores.

Computes out = x @ (W * M)^T for
  x: [16384, 2048] f32, W: [2048, 2048] f32, M: [2048, 2048] int32 (0/1)

Sharding: pure data-parallel over rows of x. Each of the 8 cores gets 2048
rows of x plus a replicated copy of W and M, computes its [2048, 2048]
output shard (transposed) locally, and the host transposes + concatenates.
No collectives.

Device-side design (~250us HW, vs 218.5us PE-streaming floor):
 - Operands in bf16 (host casts x and W; mask as int8). bf16 matmul
   streams at 1 row/cycle and its 2-byte LD_WEIGHTS fully hides under
   the previous matmul's 512-row stream (f32r's 4-byte load does not),
   so the steady-state matmul period is the 216ns floor (512 rows @
   2.4GHz). PSUM accumulates f32; rel err ~2e-3.
 - Orientation: y^T = (W*M) @ x^T. Stationary operand = [128,128] piece
   of the masked weight, moving operand = 512-row chunk of x^T; a
   [128,512] PSUM group accumulates over the 16 k-tiles (in any k
   order - groups only need start on their first and stop on their
   last accumulate).
 - The startup is per-core HBM-bound (~358 GB/s): x (8.4MB) plus
   panel-0 W/M (3.15MB) must land while the PE works. The opening
   phase runs k-major over x chunks 0+1 and all 4 sub-tiles (8 PSUM
   groups), which halves bytes-per-flop vs a single-chunk phase
   (~260 GB/s demand vs ~380); the mc1 groups join at k4 with rotated
   k order so the first k-window needs only chunk 0's first granule.
   Chunks 2,3 then run on the resident panel-0 weights, and panels
   1-3 run sub-major on the fully-resident x. All tensors are host
   pre-tiled so every DMA moves 2-4KB contiguous per partition, in
   exactly the consumption order.
 - Queue discipline, learned the hard way: every dma_start is ~0.6us
   of its engine's in-order sequencer queue, and one that WAITS (ring
   tracking, staging WAR) blocks everything behind it. So: x granules
   are batched 4 k-slices per trigger; masks ride the SWDGE ring with
   x (nothing PE-critical queues there after x lands); W rides sync;
   ALL evac copies ride ACT, whose queue carries nothing that can
   wait; y stores ride sync (plus ACT only for the final drain).
   Staging pools are sized at one panel (bufs=4) so panel t+1's
   triggers fire only as panel t's mults retire - throttling prefetch
   away from the opening phase's HBM-critical window.
 - A memzero (on ACT) + 14 warm-up matmuls on a scratch tile emitted
   first keep the PE busy from ~8us, so the clock p-state is fully
   ramped when the first real weights land (~15us; the chain
   preamble -> W/M DMA -> DVE mult -> PE has ~2us of semaphore
   latency per hop). The DVE does only mask multiplies.
"""

from contextlib import ExitStack

import ml_dtypes
import numpy as np

import concourse.bacc as bacc
import concourse.bass as bass
import concourse.mybir as mybir
import concourse.tile as tile
from concourse.bass_utils import run_bass_kernel_spmd

N_CORES = 8
P = 128

FULL_N, FULL_OUT, FULL_IN = 16384, 2048, 2048


def build_nc(
    rows: int = FULL_N // N_CORES,
    in_dim: int = FULL_IN,
    out_dim: int = FULL_OUT,
    n_panel: int = 512,
    warm_mms: int = 14,
):
    """Per-core Bass module: yt[out, rows] = (wt * m) contracted with x.

    DRAM layouts: x chunk-major pre-tiled [P, MC*KT*512] bf16; wt/mk
    pre-tiled [NTP, P, KT*n_panel] (bf16 / int8); yt [out_dim, rows] f32.
    """
    assert rows % 512 == 0 and in_dim % P == 0 and out_dim % n_panel == 0
    KT = in_dim // P  # 16 k-tiles
    NTP = out_dim // n_panel  # 4 weight panels
    SUBS = n_panel // P  # 4 stationary sub-tiles per panel
    MC = rows // 512  # 4 moving row-chunks
    KQ = 4  # k-tiles per W/mask DMA piece
    NKQ = KT // KQ
    FW = KQ * n_panel  # flat free width of a W/mask piece

    bf16 = mybir.dt.bfloat16

    nc = bacc.Bacc("TRN2", target_bir_lowering=False, debug=False)
    x = nc.dram_tensor("x", [P, MC * KT * 512], bf16, kind="ExternalInput")
    wt = nc.dram_tensor("wt", [NTP, P, KT * n_panel], bf16, kind="ExternalInput")
    mk = nc.dram_tensor("mk", [NTP, P, KT * n_panel], mybir.dt.int8, kind="ExternalInput")
    yt = nc.dram_tensor("yt", [out_dim, rows], mybir.dt.float32, kind="ExternalOutput")

    with ExitStack() as ctx:
        tc = ctx.enter_context(tile.TileContext(nc))
        xt_pool = ctx.enter_context(tc.tile_pool(name="xt", bufs=1))
        wm_pool = ctx.enter_context(tc.tile_pool(name="wm", bufs=1))
        # Staging depth 4 = one panel in flight. The WAR on a reused buffer
        # makes panel t+1's triggers fire only as panel t's mults retire —
        # a natural throttle that keeps prefetch traffic from competing
        # with the opening phase's HBM-critical x stream. (Only mask/x
        # triggers on gpsimd and W/y on sync sit behind these waits;
        # nothing PE-critical does.)
        ws_pool = ctx.enter_context(tc.tile_pool(name="ws", bufs=4))
        msk_pool = ctx.enter_context(tc.tile_pool(name="msk", bufs=4))
        yo_pool = ctx.enter_context(tc.tile_pool(name="yo", bufs=8))
        wrm_pool = ctx.enter_context(tc.tile_pool(name="wrm", bufs=1))
        pm_pool = ctx.enter_context(tc.tile_pool(name="pm", bufs=1, space="PSUM"))

        # Warm-up first in emission order: memzero has no deps, so the
        # scheduler can start it (and the warm matmuls) immediately. It
        # runs on ACT so the DVE queue is mask-multiplies-only.
        warm = wrm_pool.tile([P, 512], bf16, tag="warm", name="warm")
        nc.scalar.memzero(warm[:])
        wpm = pm_pool.tile([P, 512], mybir.dt.float32, tag="pm7", name="pmw")
        for _ in range(warm_mms):
            nc.tensor.matmul(wpm[:], warm[:, :P], warm[:], start=True, stop=True)

        # Resident x^T, chunk-major flat: slice (mc, k) at (mc*KT+k)*512.
        xt = xt_pool.tile([P, MC * KT * 512], bf16, tag="xt", name="xt")

        def xsl(mc, k):
            return slice((mc * KT + k) * 512, (mc * KT + k + 1) * 512)

        def load_x_granule(mc, k0, nk):
            # One SWDGE trigger per nk k-slices: triggers cost ~0.6us of
            # GPSIMD queue time each, and the flat chunk-major layout makes
            # the granule contiguous (nk*1KB lines per partition).
            gsl = slice((mc * KT + k0) * 512, (mc * KT + k0 + nk) * 512)
            nc.gpsimd.dma_start(out=xt[:, gsl], in_=x[:, gsl])

        # Masked-weight tiles: one per (panel, kq), [P, KQ*n_panel] bf16
        # flat, double-buffered across panels via the tag's t%2.
        wm_t = [
            [
                wm_pool.tile([P, FW], bf16, tag=f"wm{t % 2}_{q}", name=f"wm{t}_{q}")
                for q in range(NKQ)
            ]
            for t in range(NTP)
        ]

        def load_w_piece(t, q, dma_split, mult_split=None):
            """dma_split: DMA triggers per piece (each ~0.6us of queue time);
            mult_split: DVE multiplies per piece (>= dma_split shortens the
            first k-window's readiness without extra triggers)."""
            mult_split = mult_split or dma_split
            wstage = ws_pool.tile([P, FW], bf16, tag="ws")
            mtile = msk_pool.tile([P, FW], mybir.dt.int8, tag="mt")
            cw = FW // dma_split
            for c in range(dma_split):
                csl = slice(c * cw, (c + 1) * cw)
                dsl = slice(q * FW + c * cw, q * FW + (c + 1) * cw)
                nc.sync.dma_start(out=wstage[:, csl], in_=wt[t, :, dsl])
                # Masks ride the SWDGE ring: a DMA trigger that waits (ring
                # tracking, staging WAR) stalls its whole in-order engine
                # queue, and gpsimd is the only queue with nothing
                # PE-critical behind it after the x granules land. The ACT
                # queue stays copies-only so PSUM banks always drain.
                nc.gpsimd.dma_start(out=mtile[:, csl], in_=mk[t, :, dsl])
                mw = cw // (mult_split // dma_split)
                for m0 in range(c * cw, (c + 1) * cw, mw):
                    msl = slice(m0, m0 + mw)
                    nc.vector.tensor_mul(
                        wm_t[t][q][:, msl], wstage[:, msl], mtile[:, msl]
                    )

        # DMA issue order == consumption order, W-leaning: each wm piece
        # needs its DVE mult after the W DMA, so W pieces lead their
        # k-window's x granules (chunks 0 AND 1 — the opening phase runs
        # k-major over both, halving bytes-per-flop vs a single chunk).
        load_x_granule(0, 0, 2)
        load_w_piece(0, 0, 2, 4)
        load_x_granule(0, 2, 2)
        load_w_piece(0, 1, 2)
        load_x_granule(0, 4, KQ)
        load_x_granule(1, 4, KQ)
        load_w_piece(0, 2, 2)
        load_x_granule(0, 8, KQ)
        load_x_granule(1, 8, KQ)
        load_w_piece(0, 3, 2)
        load_x_granule(0, 12, KQ)
        load_x_granule(1, 12, KQ)
        load_x_granule(1, 0, KQ)  # mc1's rotated k0-3 tail is consumed last
        # Panel 1 then chunks 2-3: panel-1 triggers wait on panel-0 staging
        # WARs, so everything from here is throttled behind the opening.
        for q in range(NKQ):
            load_w_piece(1, q, 1)
        for mc in range(2, MC):
            for k0 in range(0, KT, KQ):
                load_x_granule(mc, k0, KQ)

        evac_n = 0

        def evac(pm, t, sub, mc, last=False):
            # Copies on ACT (DVE only for the final drain); y stores ALL on
            # the sync ring so no y-trigger ring-drain can ever sit ahead of
            # a copy in the ACT queue and stall a PSUM bank handoff.
            nonlocal evac_n
            yo = yo_pool.tile([P, 512], mybir.dt.float32, tag="yo")
            if last and mc % 2 == 1:
                nc.vector.tensor_copy(yo[:], pm[:])
            else:
                nc.scalar.copy(yo[:], pm[:])
            evac_n += 1
            # Final drain splits across both HWDGE rings (the ACT ring is
            # otherwise DMA-free, so no drain can block the copies).
            ydma = nc.scalar.dma_start if (last and mc % 2 == 1) else nc.sync.dma_start
            ydma(
                out=yt[(t * SUBS + sub) * P : (t * SUBS + sub + 1) * P, bass.ts(mc, 512)],
                in_=yo[:],
            )

        def pm_tile(bank):
            return pm_pool.tile(
                [P, 512], mybir.dt.float32, tag=f"pm{bank}", name=f"pm{bank}"
            )

        def mm(pm, t, sub, mc, k, start=None, stop=None):
            q, kk = k // KQ, k % KQ
            nc.tensor.matmul(
                pm[:],
                wm_t[t][q][:, kk * n_panel + sub * P : kk * n_panel + (sub + 1) * P],
                xt[:, xsl(mc, k)],
                start=(k == 0) if start is None else start,
                stop=(k == KT - 1) if stop is None else stop,
            )

        # Panel 0, opening phase: k-major over x chunks 0,1 and all subs
        # (8 live groups; bank mc*4+sub). Spreading the x-chunk cost over
        # 2x the flops keeps phase demand (~260 GB/s) under the per-core
        # HBM rate, which a single-chunk phase (~380 GB/s) exceeds. The
        # mc1 groups accumulate k in rotated order (k4..15 then k0..3, a
        # PSUM group is k-order-agnostic), so the first k-window only
        # needs chunk 0's granule and the phase ramps with the stream.
        pmsA = {(sub, mc): pm_tile(mc * 4 + sub) for sub in range(SUBS) for mc in (0, 1)}
        for k in range(KT):
            for sub in range(SUBS):
                mm(pmsA[(sub, 0)], 0, sub, 0, k)
                if k >= KQ:
                    mm(pmsA[(sub, 1)], 0, sub, 1, k, start=(k == KQ), stop=False)
        for k in range(KQ):
            for sub in range(SUBS):
                mm(pmsA[(sub, 1)], 0, sub, 1, k, start=False, stop=(k == KQ - 1))
        for mc in (0, 1):
            for sub in range(SUBS):
                evac(pmsA[(sub, mc)], 0, sub, mc)
        # Chunks 2,3 on the now-resident panel-0 weights; phase mc uses the
        # banks of opening-phase chunk mc-2, in its evac order.
        for mc in range(2, MC):
            pms = {sub: pm_tile((mc - 2) * 4 + sub) for sub in range(SUBS)}
            for k in range(KT):
                for sub in range(SUBS):
                    mm(pms[sub], 0, sub, mc, k)
            for sub in range(SUBS):
                evac(pms[sub], 0, sub, mc)

        # Panels 1-3: sub-major sweeps; sub -> banks (sub%2)*4+mc.
        for t in range(1, NTP):
            if t + 1 <= NTP - 1:
                for q in range(NKQ):
                    load_w_piece(t + 1, q, 1)
            for sub in range(SUBS):
                last = t == NTP - 1 and sub == SUBS - 1
                pms = {mc: pm_tile((sub % 2) * 4 + mc) for mc in range(MC)}
                for k in range(KT):
                    for mc in range(MC):
                        mm(pms[mc], t, sub, mc, k)
                for mc in range(MC):
                    evac(pms[mc], t, sub, mc, last=last)

    nc.compile()
    return nc


def _prep_host(input_, weight, mask, n_panel=512):
    in_dim, out_dim = weight.shape[1], weight.shape[0]
    kt = in_dim // P
    ntp = out_dim // n_panel
    # Pre-tiled [t, p, kt*n]: wtp[t, p, kt*n_panel + n] = W^T[kt*P+p, t*n_panel+n]
    wtp = np.ascontiguousarray(
        weight.T.reshape(kt, P, ntp, n_panel).transpose(2, 1, 0, 3).reshape(
            ntp, P, kt * n_panel
        )
    ).astype(ml_dtypes.bfloat16)
    mkp = np.ascontiguousarray(
        mask.T.reshape(kt, P, ntp, n_panel).transpose(2, 1, 0, 3).reshape(
            ntp, P, kt * n_panel
        )
    ).astype(np.int8)
    rows = input_.shape[0] // N_CORES
    mc = rows // 512
    in_maps = []
    for c in range(N_CORES):
        # x chunk-major: xp[p, (mc*kt + k)*512 + m] = x^T[k*P+p, mc*512+m]
        xp = np.ascontiguousarray(
            input_[c * rows : (c + 1) * rows]
            .T.reshape(kt, P, mc, 512)
            .transpose(1, 2, 0, 3)
            .reshape(P, mc * kt * 512)
        ).astype(ml_dtypes.bfloat16)
        in_maps.append({"x": xp, "wt": wtp, "mk": mkp})
    return in_maps


_CACHE = {}


def _run(input_, weight, mask, trace=False, **build_kw):
    rows_total, in_dim = input_.shape
    out_dim = weight.shape[0]
    key = (rows_total, in_dim, out_dim, tuple(sorted(build_kw.items())))
    if key not in _CACHE:
        _CACHE[key] = build_nc(
            rows=rows_total // N_CORES, in_dim=in_dim, out_dim=out_dim, **build_kw
        )
    nc = _CACHE[key]
    in_maps = _prep_host(input_, weight, mask, build_kw.get("n_panel", 512))
    res = run_bass_kernel_spmd(nc, in_maps, core_ids=list(range(N_CORES)), trace=trace)
    out = np.concatenate(
        [np.ascontiguousarray(res.results[c]["yt"].T) for c in range(N_CORES)], axis=0
    )
    return out, res


def kernel(input_, weight, mask):
    input_ = np.asarray(input_, dtype=np.float32)
    weight = np.asarray(weight, dtype=np.float32)
    mask = np.asarray(mask)
    out, _ = _run(input_, weight, mask, trace=False)
    return out



# revision 2
# speedup vs baseline: 1.0276x; 1.0276x over previous
"""Masked (expander) linear layer on 8 Trainium2 NeuronCores.

Computes out = x @ (W * M)^T for
  x: [16384, 2048] f32, W: [2048, 2048] f32, M: [2048, 2048] int32 (0/1)

Sharding: pure data-parallel over rows of x. Each of the 8 cores gets 2048
rows of x plus a replicated copy of the masked weight, computes its
[2048, 2048] output shard (transposed) locally, and the host transposes +
concatenates. No collectives.

Device-side design (v2, targeting ~234us; bf16 PE floor is ~218.5us at
2.4GHz plus a fixed ~6.6us framework preamble):
 - Operands in bf16; mask is applied on the host while casting W (the
   mask multiply is dtype/layout prep, 0.003% of the FLOPs). This removes
   the mask DMA stream (4.2MB/core) and the W-DMA -> DVE-mult -> PE
   dependency hop that used to stall the opening phase at W piece
   boundaries (~6us of stalls).
 - Orientation: y^T = (W*M) @ x^T. Stationary operand = [128,128] piece
   of the masked weight, moving operand = 512-row chunk of x^T; a
   [128,512] PSUM group accumulates over the 16 k-tiles.
 - Warm-up: memset on DVE (idle queue, no ACT table-load dependency) +
   warm matmuls on a scratch tile keep the PE clock ramping from ~6.5us
   so it is at full p-state when the first real weights land (~8.5us).
 - Opening phase runs k-major over x chunks 0+1 and all 4 sub-tiles
   (8 PSUM groups), which halves bytes-per-flop vs a single-chunk phase;
   the mc1 groups join at k4 with rotated k order. Chunks 2,3 then run
   on the resident panel-0 weights, and panels 1-3 run sub-major on the
   fully-resident x. All tensors are host pre-tiled so every DMA moves
   1-4KB contiguous per partition, in exactly the consumption order.
 - Queue discipline: every dma_start is ~0.6us of its engine's in-order
   sequencer queue, and one that WAITS (ring tracking, staging WAR)
   blocks everything behind it. x granules ride the SWDGE ring (gpsimd),
   now their sole user; W pieces ride sync, DMA'd directly into the
   double-buffered (panel parity) weight tiles - their WAR against the
   panel t-1 matmuls is already satisfied when the queue reaches them;
   evac copies ride ACT; y stores ride sync (plus scalar for the final
   drain).
 - Tail: the last sub-sweep runs group-major (per x-chunk) so three of
   its four PSUM groups evac + store while the PE still works; only the
   final group's evac (~0.7us copy + 0.7us DMA) remains after the last
   matmul, vs ~7.8us for the old all-at-the-end drain.
"""

from contextlib import ExitStack

import ml_dtypes
import numpy as np

import concourse.bacc as bacc
import concourse.bass as bass
import concourse.mybir as mybir
import concourse.tile as tile
from concourse.bass_utils import run_bass_kernel_spmd

N_CORES = 8
P = 128

FULL_N, FULL_OUT, FULL_IN = 16384, 2048, 2048


def build_nc(
    rows: int = FULL_N // N_CORES,
    in_dim: int = FULL_IN,
    out_dim: int = FULL_OUT,
    n_panel: int = 512,
    warm_mms: int = 10,
):
    """Per-core Bass module: yt[out, rows] = wt contracted with x.

    DRAM layouts: x chunk-major pre-tiled [P, MC*KT*512] bf16; wt (already
    masked) pre-tiled [NTP, P, KT*n_panel] bf16; yt [out_dim, rows] f32.
    """
    assert rows % 512 == 0 and in_dim % P == 0 and out_dim % n_panel == 0
    KT = in_dim // P  # 16 k-tiles
    NTP = out_dim // n_panel  # 4 weight panels
    SUBS = n_panel // P  # 4 stationary sub-tiles per panel
    MC = rows // 512  # 4 moving row-chunks
    KQ = 4  # k-tiles per W DMA piece
    NKQ = KT // KQ
    FW = KQ * n_panel  # flat free width of a W piece

    bf16 = mybir.dt.bfloat16

    nc = bacc.Bacc("TRN2", target_bir_lowering=False, debug=False)
    x = nc.dram_tensor("x", [P, MC * KT * 512], bf16, kind="ExternalInput")
    wt = nc.dram_tensor("wt", [NTP, P, KT * n_panel], bf16, kind="ExternalInput")
    yt = nc.dram_tensor("yt", [out_dim, rows], mybir.dt.float32, kind="ExternalOutput")

    with ExitStack() as ctx:
        tc = ctx.enter_context(tile.TileContext(nc))
        xt_pool = ctx.enter_context(tc.tile_pool(name="xt", bufs=1))
        wm_pool = ctx.enter_context(tc.tile_pool(name="wm", bufs=1))
        yo_pool = ctx.enter_context(tc.tile_pool(name="yo", bufs=8))
        wrm_pool = ctx.enter_context(tc.tile_pool(name="wrm", bufs=1))
        pm_pool = ctx.enter_context(tc.tile_pool(name="pm", bufs=1, space="PSUM"))

        # Warm-up first in emission order. The memset rides the otherwise
        # idle DVE queue (no ACT table-load dependency), so warm matmuls
        # start right after the framework preamble (~6.5us) and the PE
        # clock is fully ramped when the first real weights land.
        warm = wrm_pool.tile([P, 512], bf16, tag="warm", name="warm")
        nc.vector.memset(warm[:], 0)
        wpm = pm_pool.tile([P, 512], mybir.dt.float32, tag="pm7", name="pmw")
        for _ in range(warm_mms):
            nc.tensor.matmul(wpm[:], warm[:, :P], warm[:], start=True, stop=True)

        # Resident x^T, chunk-major flat: slice (mc, k) at (mc*KT+k)*512.
        xt = xt_pool.tile([P, MC * KT * 512], bf16, tag="xt", name="xt")

        def xsl(mc, k):
            return slice((mc * KT + k) * 512, (mc * KT + k + 1) * 512)

        def load_x_granule(mc, k0, nk):
            # One SWDGE trigger per nk k-slices: triggers cost ~0.6us of
            # GPSIMD queue time each, and the flat chunk-major layout makes
            # the granule contiguous (nk*1KB lines per partition).
            gsl = slice((mc * KT + k0) * 512, (mc * KT + k0 + nk) * 512)
            nc.gpsimd.dma_start(out=xt[:, gsl], in_=x[:, gsl])

        # Masked-weight tiles: one per (panel, kq), [P, KQ*n_panel] bf16
        # flat, double-buffered across panels via the tag's t%2. W DMAs
        # land directly in these (host pre-masks); the WAR against panel
        # t-2's matmuls is long satisfied when the sync queue reaches the
        # prefetch triggers, so they never stall the ring.
        wm_t = [
            [
                wm_pool.tile([P, FW], bf16, tag=f"wm{t % 2}_{q}", name=f"wm{t}_{q}")
                for q in range(NKQ)
            ]
            for t in range(NTP)
        ]

        def load_w_piece(t, q, dma_split):
            """dma_split: DMA triggers per piece (each ~0.6us of queue
            time); a split of s makes the first k-tiles of the piece
            PE-ready after 1/s of the piece's HBM time."""
            cw = FW // dma_split
            for c in range(dma_split):
                csl = slice(c * cw, (c + 1) * cw)
                dsl = slice(q * FW + c * cw, q * FW + (c + 1) * cw)
                nc.sync.dma_start(out=wm_t[t][q][:, csl], in_=wt[t, :, dsl])

        # DMA issue order == consumption order, W-leaning: W pieces lead
        # their k-window's x granules (chunks 0 AND 1 - the opening phase
        # runs k-major over both, halving bytes-per-flop vs one chunk).
        load_w_piece(0, 0, 4)
        load_x_granule(0, 0, 1)
        load_x_granule(0, 1, 1)
        load_x_granule(0, 2, 2)
        load_w_piece(0, 1, 2)
        load_x_granule(0, 4, KQ)
        load_x_granule(1, 4, KQ)
        load_w_piece(0, 2, 2)
        load_x_granule(0, 8, KQ)
        load_x_granule(1, 8, KQ)
        load_w_piece(0, 3, 2)
        load_x_granule(0, 12, KQ)
        load_x_granule(1, 12, KQ)
        load_x_granule(1, 0, KQ)  # mc1's rotated k0-3 tail is consumed last
        for q in range(NKQ):
            load_w_piece(1, q, 1)
        for mc in range(2, MC):
            for k0 in range(0, KT, KQ):
                load_x_granule(mc, k0, KQ)

        def evac(pm, t, sub, mc, last=False):
            # Copies on ACT; y stores on the sync ring, except the very
            # last group's store which rides the otherwise-empty scalar
            # ring so it starts the moment its copy retires.
            yo = yo_pool.tile([P, 512], mybir.dt.float32, tag="yo")
            nc.scalar.copy(yo[:], pm[:])
            ydma = nc.scalar.dma_start if last else nc.sync.dma_start
            ydma(
                out=yt[(t * SUBS + sub) * P : (t * SUBS + sub + 1) * P, bass.ts(mc, 512)],
                in_=yo[:],
            )

        def pm_tile(bank):
            return pm_pool.tile(
                [P, 512], mybir.dt.float32, tag=f"pm{bank}", name=f"pm{bank}"
            )

        def mm(pm, t, sub, mc, k, start=None, stop=None):
            q, kk = k // KQ, k % KQ
            nc.tensor.matmul(
                pm[:],
                wm_t[t][q][:, kk * n_panel + sub * P : kk * n_panel + (sub + 1) * P],
                xt[:, xsl(mc, k)],
                start=(k == 0) if start is None else start,
                stop=(k == KT - 1) if stop is None else stop,
            )

        # Panel 0, opening phase: k-major over x chunks 0,1 and all subs
        # (8 live groups; bank mc*4+sub). Spreading the x-chunk cost over
        # 2x the flops keeps phase HBM demand (~225 GB/s) under the
        # per-core HBM rate. The mc1 groups accumulate k in rotated order
        # (k4..15 then k0..3 - a PSUM group is k-order-agnostic), so the
        # first k-window only needs chunk 0's granule.
        pmsA = {(sub, mc): pm_tile(mc * 4 + sub) for sub in range(SUBS) for mc in (0, 1)}
        for k in range(KT):
            for sub in range(SUBS):
                mm(pmsA[(sub, 0)], 0, sub, 0, k)
                if k >= KQ:
                    mm(pmsA[(sub, 1)], 0, sub, 1, k, start=(k == KQ), stop=False)
        for k in range(KQ):
            for sub in range(SUBS):
                mm(pmsA[(sub, 1)], 0, sub, 1, k, start=False, stop=(k == KQ - 1))
        for mc in (0, 1):
            for sub in range(SUBS):
                evac(pmsA[(sub, mc)], 0, sub, mc)
        # Chunks 2,3 on the now-resident panel-0 weights; phase mc uses the
        # banks of opening-phase chunk mc-2, in its evac order.
        for mc in range(2, MC):
            pms = {sub: pm_tile((mc - 2) * 4 + sub) for sub in range(SUBS)}
            for k in range(KT):
                for sub in range(SUBS):
                    mm(pms[sub], 0, sub, mc, k)
            for sub in range(SUBS):
                evac(pms[sub], 0, sub, mc)

        # Panels 1-3: sub-major sweeps; sub -> banks (sub%2)*4+mc. The
        # final sub-sweep runs group-major (per mc) so its evacs overlap
        # the remaining matmuls and only the last group drains after the
        # final matmul.
        for t in range(1, NTP):
            if t + 1 <= NTP - 1:
                for q in range(NKQ):
                    load_w_piece(t + 1, q, 1)
            for sub in range(SUBS):
                final_sweep = t == NTP - 1 and sub == SUBS - 1
                pms = {mc: pm_tile((sub % 2) * 4 + mc) for mc in range(MC)}
                if final_sweep:
                    for mc in range(MC):
                        for k in range(KT):
                            mm(pms[mc], t, sub, mc, k)
                        evac(pms[mc], t, sub, mc, last=(mc == MC - 1))
                else:
                    for k in range(KT):
                        for mc in range(MC):
                            mm(pms[mc], t, sub, mc, k)
                    for mc in range(MC):
                        evac(pms[mc], t, sub, mc)

    nc.compile()
    return nc


def _prep_host(input_, weight, mask, n_panel=512):
    in_dim, out_dim = weight.shape[1], weight.shape[0]
    kt = in_dim // P
    ntp = out_dim // n_panel
    masked = (weight * mask.astype(weight.dtype)).astype(np.float32)
    # Pre-tiled [t, p, kt*n]: wtp[t, p, kt*n_panel + n] = Wm^T[kt*P+p, t*n_panel+n]
    wtp = np.ascontiguousarray(
        masked.T.reshape(kt, P, ntp, n_panel).transpose(2, 1, 0, 3).reshape(
            ntp, P, kt * n_panel
        )
    ).astype(ml_dtypes.bfloat16)
    rows = input_.shape[0] // N_CORES
    mc = rows // 512
    in_maps = []
    for c in range(N_CORES):
        # x chunk-major: xp[p, (mc*kt + k)*512 + m] = x^T[k*P+p, mc*512+m]
        xp = np.ascontiguousarray(
            input_[c * rows : (c + 1) * rows]
            .T.reshape(kt, P, mc, 512)
            .transpose(1, 2, 0, 3)
            .reshape(P, mc * kt * 512)
        ).astype(ml_dtypes.bfloat16)
        in_maps.append({"x": xp, "wt": wtp})
    return in_maps


_CACHE = {}


def _run(input_, weight, mask, trace=False, **build_kw):
    rows_total, in_dim = input_.shape
    out_dim = weight.shape[0]
    key = (rows_total, in_dim, out_dim, tuple(sorted(build_kw.items())))
    if key not in _CACHE:
        _CACHE[key] = build_nc(
            rows=rows_total // N_CORES, in_dim=in_dim, out_dim=out_dim, **build_kw
        )
    nc = _CACHE[key]
    in_maps = _prep_host(input_, weight, mask, build_kw.get("n_panel", 512))
    res = run_bass_kernel_spmd(nc, in_maps, core_ids=list(range(N_CORES)), trace=trace)
    out = np.concatenate(
        [np.ascontiguousarray(res.results[c]["yt"].T) for c in range(N_CORES)], axis=0
    )
    return out, res


def kernel(input_, weight, mask):
    input_ = np.asarray(input_, dtype=np.float32)
    weight = np.asarray(weight, dtype=np.float32)
    mask = np.asarray(mask)
    out, _ = _run(input_, weight, mask, trace=False)
    return out


# revision 3
# speedup vs baseline: 1.0865x; 1.0573x over previous
"""Masked (expander) linear layer on 8 Trainium2 NeuronCores.

Computes out = x @ (W * M)^T for
  x: [16384, 2048] f32, W: [2048, 2048] f32, M: [2048, 2048] int32 (0/1)

Sharding: pure data-parallel over rows of x. Each of the 8 cores gets 2048
rows of x plus a replicated copy of the masked weight, computes its
[2048, 2048] output shard (transposed) locally, and the host transposes +
concatenates. No collectives.

Device-side design (v3):
 - Orientation: y^T = (W*M) @ x^T. Stationary operand = [128,128] piece
   of the masked weight, moving operand = 512-row chunk of x^T; a
   [128,512] PSUM group accumulates over the contraction.
 - Mixed precision: k-tiles 0-13 run in bf16 (1 row/cycle); k-tiles
   14-15 run as ONE fp8e4 DoubleRow matmul (two fp8 k-tiles contracted
   per pass at the same per-pass cost, i.e. 2x FLOPs - measured on HW).
   That cuts the pass count per group from 16 to 15 (-6.2% PE time) for
   a rel err of ~1.3e-2 on the reference inputs (gate 2e-2; fp8 on 1/8
   of the contraction contributes sqrt(1/8)*3.9e-2). The mask is applied
   on the host while casting W (dtype/layout prep; 0.003% of the FLOPs),
   which also removes the mask DMA stream and the DVE hop from the
   W-ready critical path.
 - The opening phase is chip-HBM-bound (all 8 cores pull x + replicated
   W concurrently at the ~3TB/s chip roofline, and the DMA pipe only
   reaches full rate ~6us in). So the opening runs k-major over x
   chunks 0+1 and all 4 sub-tiles (8 PSUM groups), halving bytes-per-
   flop vs a single-chunk phase, and the mc0 groups run their fp8
   DoubleRow pass FIRST: the first PE windows then need half the bytes
   (fp8) right when HBM is slowest. mc1 joins at k4 with rotated k
   order; its DoubleRow pass comes last. Chunks 2,3 then run on the
   resident panel-0 weights, and panels 1-3 run sub-major on the fully-
   resident x. All tensors are host pre-tiled so every DMA moves 1-4KB
   contiguous per partition, in exactly the consumption order.
 - Warm-up: memset on DVE (idle queue, no ACT table-load dependency) +
   warm matmuls on a scratch tile ramp the PE clock from ~6.5us (after
   the fixed ~6.6us framework preamble) so it is near full p-state when
   the first real operands land (~9.5us).
 - Queue discipline: every dma_start is ~0.6us of its engine's in-order
   sequencer queue, and one that WAITS blocks everything behind it.
   x granules ride the SWDGE ring (gpsimd), their sole user; W pieces
   ride sync, DMA'd directly into the double-buffered (panel parity)
   weight tiles - their WAR against panel t-1's matmuls is already
   satisfied when the queue reaches them; evac copies ride ACT; y
   stores ride sync (plus scalar for the final drain).
 - Tail: the last sub-sweep runs group-major (per x-chunk) so three of
   its four PSUM groups evac + store while the PE still works; only the
   final group's evac (~0.7us copy + 0.7us DMA) remains after the last
   matmul.
"""

from contextlib import ExitStack

import ml_dtypes
import numpy as np

import concourse.bacc as bacc
import concourse.bass as bass
import concourse.mybir as mybir
import concourse.tile as tile
from concourse.bass_utils import run_bass_kernel_spmd

N_CORES = 8
P = 128

FULL_N, FULL_OUT, FULL_IN = 16384, 2048, 2048
KTB = 14  # k-tiles computed in bf16; the last 2 ride one fp8 DoubleRow pass


def build_nc(
    rows: int = FULL_N // N_CORES,
    in_dim: int = FULL_IN,
    out_dim: int = FULL_OUT,
    n_panel: int = 512,
    warm_mms: int = 6,
):
    """Per-core Bass module: yt[out, rows] = wt contracted with x.

    DRAM layouts (host pre-tiled, mask already applied, bf16/fp8 cast):
      x  [P, MC*KTB*512]   bf16  - k-tiles 0-13, chunk-major
      x8 [P, MC, 2, 512]   fp8e4 - k-tiles 14,15 as DoubleRow pairs
      wt [NTP, P, KTB*n_panel] bf16
      w8 [NTP, P, 2, n_panel]  fp8e4
      yt [out_dim, rows]   f32
    """
    assert rows % 512 == 0 and in_dim % P == 0 and out_dim % n_panel == 0
    KT = in_dim // P  # 16 k-tiles total
    assert KT == KTB + 2
    NTP = out_dim // n_panel  # 4 weight panels
    SUBS = n_panel // P  # 4 stationary sub-tiles per panel
    MC = rows // 512  # 4 moving row-chunks
    KQ = 4  # k-tiles per full W DMA piece
    WQ = [(0, 4), (4, 4), (8, 4), (12, 2)]  # (k0, nk) per piece
    NKQ = len(WQ)

    bf16 = mybir.dt.bfloat16
    fp8 = mybir.dt.float8e4
    DR = mybir.MatmulPerfMode.DoubleRow

    nc = bacc.Bacc("TRN2", target_bir_lowering=False, debug=False)
    x = nc.dram_tensor("x", [P, MC * KTB * 512], bf16, kind="ExternalInput")
    x8 = nc.dram_tensor("x8", [P, MC, 2, 512], fp8, kind="ExternalInput")
    wt = nc.dram_tensor("wt", [NTP, P, KTB * n_panel], bf16, kind="ExternalInput")
    w8 = nc.dram_tensor("w8", [NTP, P, 2, n_panel], fp8, kind="ExternalInput")
    yt = nc.dram_tensor("yt", [out_dim, rows], mybir.dt.float32, kind="ExternalOutput")

    with ExitStack() as ctx:
        tc = ctx.enter_context(tile.TileContext(nc))
        xt_pool = ctx.enter_context(tc.tile_pool(name="xt", bufs=1))
        wm_pool = ctx.enter_context(tc.tile_pool(name="wm", bufs=1))
        yo_pool = ctx.enter_context(tc.tile_pool(name="yo", bufs=8))
        wrm_pool = ctx.enter_context(tc.tile_pool(name="wrm", bufs=1))
        pm_pool = ctx.enter_context(tc.tile_pool(name="pm", bufs=1, space="PSUM"))

        # Warm-up first in emission order; memset rides the idle DVE queue.
        warm = wrm_pool.tile([P, 512], bf16, tag="warm", name="warm")
        nc.vector.memset(warm[:], 0)
        wpm = pm_pool.tile([P, 512], mybir.dt.float32, tag="pm7", name="pmw")
        for _ in range(warm_mms):
            nc.tensor.matmul(wpm[:], warm[:, :P], warm[:], start=True, stop=True)

        # Resident x^T k0-13, chunk-major flat; fp8 pairs in their own tile.
        xt = xt_pool.tile([P, MC * KTB * 512], bf16, tag="xt", name="xt")
        x8t = xt_pool.tile([P, MC, 2, 512], fp8, tag="x8t", name="x8t")

        def xsl(mc, k):
            return slice((mc * KTB + k) * 512, (mc * KTB + k + 1) * 512)

        def load_x_granule(mc, k0, nk):
            gsl = slice((mc * KTB + k0) * 512, (mc * KTB + k0 + nk) * 512)
            nc.gpsimd.dma_start(out=xt[:, gsl], in_=x[:, gsl])

        def load_x8_granule(mc):
            nc.gpsimd.dma_start(out=x8t[:, mc], in_=x8[:, mc])

        # Masked-weight tiles: bf16 per (panel, piece) + one fp8 pair tile
        # per panel, double-buffered across panels via the tag's t%2.
        wm_t = [
            [
                wm_pool.tile(
                    [P, nk * n_panel], bf16, tag=f"wm{t % 2}_{q}", name=f"wm{t}_{q}"
                )
                for q, (k0, nk) in enumerate(WQ)
            ]
            for t in range(NTP)
        ]
        w8_t = [
            wm_pool.tile([P, 2, n_panel], fp8, tag=f"w8{t % 2}", name=f"w8{t}")
            for t in range(NTP)
        ]

        def load_w_piece(t, q, dma_split):
            k0, nk = WQ[q]
            fw = nk * n_panel
            cw = fw // dma_split
            for c in range(dma_split):
                csl = slice(c * cw, (c + 1) * cw)
                dsl = slice(k0 * n_panel + c * cw, k0 * n_panel + (c + 1) * cw)
                nc.sync.dma_start(out=wm_t[t][q][:, csl], in_=wt[t, :, dsl])

        def load_w8(t):
            nc.sync.dma_start(out=w8_t[t][:], in_=w8[t])

        # DMA issue order == consumption order. The fp8 pair tiles lead:
        # the opening's first PE windows are the mc0 DoubleRow passes.
        load_w8(0)
        load_x8_granule(0)
        load_w_piece(0, 0, 4)
        load_x_granule(0, 0, 1)
        load_x_granule(0, 1, 1)
        load_x_granule(0, 2, 2)
        load_w_piece(0, 1, 2)
        load_x_granule(0, 4, KQ)
        load_x_granule(1, 4, KQ)
        load_w_piece(0, 2, 2)
        load_x_granule(0, 8, KQ)
        load_x_granule(1, 8, KQ)
        load_w_piece(0, 3, 1)
        load_x_granule(0, 12, 2)
        load_x_granule(1, 12, 2)
        load_x_granule(1, 0, KQ)  # mc1's rotated k0-3 tail
        load_x8_granule(1)  # mc1's DoubleRow pass is its last
        for q in range(NKQ):
            load_w_piece(1, q, 1)
        load_w8(1)
        for mc in range(2, MC):
            for k0 in range(0, KTB, KQ):
                load_x_granule(mc, k0, min(KQ, KTB - k0))
            load_x8_granule(mc)

        def evac(pm, t, sub, mc, last=False):
            yo = yo_pool.tile([P, 512], mybir.dt.float32, tag="yo")
            nc.scalar.copy(yo[:], pm[:])
            ydma = nc.scalar.dma_start if last else nc.sync.dma_start
            ydma(
                out=yt[(t * SUBS + sub) * P : (t * SUBS + sub + 1) * P, bass.ts(mc, 512)],
                in_=yo[:],
            )

        def pm_tile(bank):
            return pm_pool.tile(
                [P, 512], mybir.dt.float32, tag=f"pm{bank}", name=f"pm{bank}"
            )

        def mm(pm, t, sub, mc, k, start, stop):
            q = min(k // KQ, NKQ - 1)
            kk = k - WQ[q][0]
            nc.tensor.matmul(
                pm[:],
                wm_t[t][q][:, kk * n_panel + sub * P : kk * n_panel + (sub + 1) * P],
                xt[:, xsl(mc, k)],
                start=start,
                stop=stop,
            )

        def dr(pm, t, sub, mc, start, stop):
            nc.tensor.matmul(
                pm[:],
                w8_t[t][:, :, sub * P : (sub + 1) * P],
                x8t[:, mc],
                start=start,
                stop=stop,
                perf_mode=DR,
            )

        # Panel 0, opening phase: k-major over x chunks 0,1 and all subs
        # (8 live groups; bank mc*4+sub). mc0 groups open with their fp8
        # DoubleRow pass (least bytes per flop, when HBM is coldest); mc1
        # joins at k4 in rotated order and closes with its DoubleRow.
        pmsA = {(sub, mc): pm_tile(mc * 4 + sub) for sub in range(SUBS) for mc in (0, 1)}
        for sub in range(SUBS):
            dr(pmsA[(sub, 0)], 0, sub, 0, start=True, stop=False)
        for k in range(KTB):
            for sub in range(SUBS):
                mm(pmsA[(sub, 0)], 0, sub, 0, k, start=False, stop=(k == KTB - 1))
                if k >= KQ:
                    mm(pmsA[(sub, 1)], 0, sub, 1, k, start=(k == KQ), stop=False)
        for sub in range(SUBS):
            evac(pmsA[(sub, 0)], 0, sub, 0)
        for k in range(KQ):
            for sub in range(SUBS):
                mm(pmsA[(sub, 1)], 0, sub, 1, k, start=False, stop=False)
        for sub in range(SUBS):
            dr(pmsA[(sub, 1)], 0, sub, 1, start=False, stop=True)
            evac(pmsA[(sub, 1)], 0, sub, 1)
        # Chunks 2,3 on the now-resident panel-0 weights; phase mc uses the
        # banks of opening-phase chunk mc-2, in its evac order.
        for mc in range(2, MC):
            pms = {sub: pm_tile((mc - 2) * 4 + sub) for sub in range(SUBS)}
            for k in range(KTB):
                for sub in range(SUBS):
                    mm(pms[sub], 0, sub, mc, k, start=(k == 0), stop=False)
            for sub in range(SUBS):
                dr(pms[sub], 0, sub, mc, start=False, stop=True)
                evac(pms[sub], 0, sub, mc)

        # Panels 1-3: sub-major sweeps; sub -> banks (sub%2)*4+mc. The
        # final sub-sweep runs group-major (per mc) so its evacs overlap
        # the remaining matmuls.
        for t in range(1, NTP):
            if t + 1 <= NTP - 1:
                for q in range(NKQ):
                    load_w_piece(t + 1, q, 1)
                load_w8(t + 1)
            for sub in range(SUBS):
                final_sweep = t == NTP - 1 and sub == SUBS - 1
                pms = {mc: pm_tile((sub % 2) * 4 + mc) for mc in range(MC)}
                if final_sweep:
                    for mc in range(MC):
                        for k in range(KTB):
                            mm(pms[mc], t, sub, mc, k, start=(k == 0), stop=False)
                        dr(pms[mc], t, sub, mc, start=False, stop=True)
                        evac(pms[mc], t, sub, mc, last=(mc == MC - 1))
                else:
                    for k in range(KTB):
                        for mc in range(MC):
                            mm(pms[mc], t, sub, mc, k, start=(k == 0), stop=False)
                    for mc in range(MC):
                        dr(pms[mc], t, sub, mc, start=False, stop=True)
                        evac(pms[mc], t, sub, mc)

    nc.compile()
    return nc


def _prep_host(input_, weight, mask, n_panel=512):
    in_dim, out_dim = weight.shape[1], weight.shape[0]
    kt = in_dim // P
    ntp = out_dim // n_panel
    masked = (weight * mask.astype(weight.dtype)).astype(np.float32)
    # masked^T tiled [kt, P, ntp, n_panel]
    wtk = masked.T.reshape(kt, P, ntp, n_panel)
    wtp = np.ascontiguousarray(
        wtk[:KTB].transpose(2, 1, 0, 3).reshape(ntp, P, KTB * n_panel)
    ).astype(ml_dtypes.bfloat16)
    w8p = np.ascontiguousarray(wtk[KTB:].transpose(2, 1, 0, 3)).astype(
        ml_dtypes.float8_e4m3
    )  # [ntp, P, 2, n_panel]
    rows = input_.shape[0] // N_CORES
    mc = rows // 512
    in_maps = []
    for c in range(N_CORES):
        xtk = input_[c * rows : (c + 1) * rows].T.reshape(kt, P, mc, 512)
        xp = np.ascontiguousarray(
            xtk[:KTB].transpose(1, 2, 0, 3).reshape(P, mc * KTB * 512)
        ).astype(ml_dtypes.bfloat16)
        x8p = np.ascontiguousarray(xtk[KTB:].transpose(1, 2, 0, 3)).astype(
            ml_dtypes.float8_e4m3
        )  # [P, mc, 2, 512]
        in_maps.append({"x": xp, "x8": x8p, "wt": wtp, "w8": w8p})
    return in_maps


_CACHE = {}


def _run(input_, weight, mask, trace=False, **build_kw):
    rows_total, in_dim = input_.shape
    out_dim = weight.shape[0]
    key = (rows_total, in_dim, out_dim, tuple(sorted(build_kw.items())))
    if key not in _CACHE:
        _CACHE[key] = build_nc(
            rows=rows_total // N_CORES, in_dim=in_dim, out_dim=out_dim, **build_kw
        )
    nc = _CACHE[key]
    in_maps = _prep_host(input_, weight, mask, build_kw.get("n_panel", 512))
    res = run_bass_kernel_spmd(nc, in_maps, core_ids=list(range(N_CORES)), trace=trace)
    out = np.concatenate(
        [np.ascontiguousarray(res.results[c]["yt"].T) for c in range(N_CORES)], axis=0
    )
    return out, res


def kernel(input_, weight, mask):
    input_ = np.asarray(input_, dtype=np.float32)
    weight = np.asarray(weight, dtype=np.float32)
    mask = np.asarray(mask)
    out, _ = _run(input_, weight, mask, trace=False)
    return out
